# revision 7
# baseline (speedup 1.0000x reference)
"""FlowNet correlation (max_disp=20, stride2=2 -> 441 channels) on 8 TRN2 cores.

Strategy
--------
Data-parallel over batch: B=16 -> 2 batches per core.

Per (batch, parity-class (y%2, x%2)) the correlation decomposes into an
independent sub-problem on a 32x48 pixel grid with 21x21 displacements into a
zero-padded 52x68 source grid:

    out[dy,dx,ye,xe] = sum_c f1s[c,ye,xe] * f2s[c, ye+dy, xe+dx]

This is computed on the PE array as banded all-pairs matmuls: stationary
operand = a block of 8x16 = 128 pixel columns of f1 (bf16), moving operand =
the (8+20)x(16+20) = 28x36 = 1008 source columns of padded f2 (bf16), PSUM
accumulates over the two 128-channel chunks.  Each output pixel's 441 useful
dot products sit in a 21x21 sub-rectangle of its PSUM row (43.75% PE
utilization).  The full band tiles are scaled by 1/C, cast to bf16, copied
PSUM->SBUF (split across Vector + Scalar engines), and DMAed contiguously to
DRAM.  The final 21x21-per-pixel shear extraction is a zero-copy
numpy as_strided view on the host, followed by one big transpose/copy.

Parities are handled with strided access patterns directly on unsplit SBUF
tensors (f1 [C,64,96], padded f2 [C,104,136]) - no physical parity split.
"""

import sys

for _p in ("/opt/trn_rl_repo", "/root/.axon_site/_ro/trn_rl_repo"):
    if _p not in sys.path:
        sys.path.insert(0, _p)

import numpy as np

import concourse.bass as bass
import concourse.bacc as bacc
import concourse.mybir as mybir
import concourse.tile as tile
from concourse.bass_utils import run_bass_kernel_spmd

# Problem constants (hardcoded per contract)
B, C, H, W = 16, 256, 64, 96
NCORES = 8
BPC = B // NCORES            # batches per core
D = 21                       # displacements per axis
PAD_V, PAD_U = 52, 68        # padded source subgrid (per parity class)
P, Q = 8, 16                 # pixel block (y_e x x_e); P*Q = 128 stationary cols
SV, SU = P + 20, Q + 20      # source block 28 x 36 = 1008 moving cols
NIY, NIX = 32 // P, 48 // Q  # 4 x 3 blocks per class
F32 = mybir.dt.float32
BF16 = mybir.dt.bfloat16

_NC_CACHE = {}


def _build_nc(rep: int = 1):
    """Build the SPMD bass program (identical on all 8 cores).

    rep > 1 repeats the compute+output loop (same outputs) for timing slopes.
    """
    nc = bacc.Bacc("TRN2", target_bir_lowering=False, debug=False)
    f1d = nc.declare_dram_parameter("input1", [BPC, C, H, W], F32, isOutput=False)
    f2d = nc.declare_dram_parameter("input2", [BPC, C, H, W], F32, isOutput=False)
    # raw band tiles: [b, cls, iy, ix, 128 pixel, 28*36 source] bf16
    raw = nc.declare_dram_parameter(
        "raw", [BPC, 4, NIY, NIX, P * Q, SV * SU], BF16, isOutput=True
    )

    with tile.TileContext(nc) as tc:
        with (
            tc.tile_pool(name="f1p", bufs=1) as f1pool,
            tc.tile_pool(name="f2p", bufs=1) as f2pool,
            tc.tile_pool(name="stage", bufs=4) as spool,
            tc.tile_pool(name="psum", bufs=4, space="PSUM") as ppool,
        ):
            # f1 raw (single-buffered; only read by the staging copies)
            f1t = [f1pool.tile([128, H, W], BF16, name=f"f1r_{k}", tag=f"f1r_{k}")
                   for k in range(2)]
            # f1 staged block-major [ix, 128 pixel cols]; one tile per
            # (buf, chunk, cls, iy) so each matmul has a single staging producer
            f1b = [[[[f1pool.tile([128, NIX, P * Q], BF16,
                                  name=f"f1b_{i}_{k}_{c}_{j}",
                                  tag=f"f1b_{i}_{k}_{c}_{j}")
                      for j in range(NIY)] for c in range(4)]
                    for k in range(2)] for i in range(2)]
            f2t = [[f2pool.tile([128, 104, 136], BF16, name=f"f2_{i}_{k}", tag=f"f2_{i}_{k}")
                    for k in range(2)] for i in range(2)]

            # zero the padded f2 buffers once; borders stay zero forever
            for i in range(2):
                for k in range(2):
                    nc.vector.memset(f2t[i][k][:], 0.0)

            def load_b(b):
                buf = b % 2
                for k in range(2):  # channel chunk
                    c0 = k * 128
                    nc.gpsimd.dma_start(
                        out=f1t[k][:], in_=f1d[b, c0:c0 + 128]
                    )
                    nc.gpsimd.dma_start(
                        out=f2t[buf][k][:, 20:84, 20:116],
                        in_=f2d[b, c0:c0 + 128],
                    )
                # stage f1 into contiguous 128-column pixel blocks
                # (matmul stationary APs must have a single free dim)
                for k in range(2):
                    f1vk = f1t[k][:].rearrange(
                        "p (y a) (x c) -> p a c y x", a=2, c=2)
                    for cls in range(4):
                        py, px = cls >> 1, cls & 1
                        for iy in range(NIY):
                            nc.gpsimd.tensor_copy(
                                f1b[buf][k][cls][iy][:]
                                .rearrange("p i (y x) -> p i y x", y=P),
                                f1vk[:, py, px, iy * P:(iy + 1) * P, :]
                                .rearrange("p y (i x) -> p i y x", i=NIX),
                            )

            load_b(0)
            if BPC > 1:
                load_b(1)

            inv_c = 1.0 / C

            for r in range(rep):
                for b in range(BPC):
                    buf = b % 2
                    # parity-interleave views: [p, py, px, ve, ue]
                    f2v = [f2t[buf][k][:].rearrange(
                        "p (v a) (u c) -> p a c v u", a=2, c=2) for k in range(2)]
                    for cls in range(4):
                        py, px = cls >> 1, cls & 1
                        for iy in range(NIY):
                            y0 = iy * P
                            for ix in range(NIX):
                                x0 = ix * Q
                                ps = [ppool.tile([128, SV // 2, SU], F32, name=f"ps{h}",
                                                 tag=f"ps{h}") for h in range(2)]
                                for h in range(2):  # v-halves of moving operand
                                    v0 = y0 + h * (SV // 2)
                                    for k in range(2):  # channel chunk
                                        nc.tensor.matmul(
                                            ps[h][:],
                                            f1b[buf][k][cls][iy][:, ix],
                                            f2v[k][:, py, px,
                                                   v0:v0 + SV // 2,
                                                   x0:x0 + SU],
                                            start=(k == 0),
                                            stop=(k == 1),
                                        )
                                st = spool.tile([128, SV * SU], BF16, name="st", tag="st")
                                half = (SV // 2) * SU
                                # split the scale+cast copy across two engines
                                nc.vector.tensor_scalar_mul(
                                    st[:, 0:half],
                                    ps[0][:].rearrange("p a b -> p (a b)"),
                                    inv_c,
                                )
                                nc.scalar.activation(
                                    st[:, half:2 * half],
                                    ps[1][:].rearrange("p a b -> p (a b)"),
                                    mybir.ActivationFunctionType.Copy,
                                    scale=inv_c,
                                )
                                nc.sync.dma_start(
                                    out=raw[b, cls, iy, ix], in_=st[:]
                                )
    nc.compile()
    return nc


def _get_nc(rep: int = 1):
    if rep not in _NC_CACHE:
        _NC_CACHE[rep] = _build_nc(rep)
    return _NC_CACHE[rep]


def _extract(raw_np: np.ndarray) -> np.ndarray:
    """Host-side shear extraction: raw [BPC,4,NIY,NIX,128,1008] -> [BPC,441,64,96]."""
    arr = np.ascontiguousarray(raw_np).astype(np.float32)
    V = arr.reshape(BPC, 4, NIY, NIX, P, Q, SV, SU)
    s = V.strides
    Wv = np.lib.stride_tricks.as_strided(
        V,
        shape=(BPC, 4, NIY, NIX, P, Q, D, D),
        strides=(s[0], s[1], s[2], s[3], s[4] + s[6], s[5] + s[7], s[6], s[7]),
    )
    # -> [b, cls, dy, dx, iy, yo, ix, xo] -> [b, cls, 441, 32, 48]
    T = Wv.transpose(0, 1, 6, 7, 2, 4, 3, 5).reshape(BPC, 4, D * D, 32, 48)
    out = np.empty((BPC, D * D, H, W), np.float32)
    for cls in range(4):
        py, px = cls >> 1, cls & 1
        out[:, :, py::2, px::2] = T[:, cls]
    return out


def kernel(input1: np.ndarray, input2: np.ndarray) -> np.ndarray:
    input1 = np.ascontiguousarray(input1, dtype=np.float32)
    input2 = np.ascontiguousarray(input2, dtype=np.float32)
    nc = _get_nc(1)
    in_maps = [
        {"input1": input1[i * BPC:(i + 1) * BPC],
         "input2": input2[i * BPC:(i + 1) * BPC]}
        for i in range(NCORES)
    ]
    res = run_bass_kernel_spmd(nc, in_maps, list(range(NCORES)))
    out = np.empty((B, D * D, H, W), np.float32)
    for i in range(NCORES):
        out[i * BPC:(i + 1) * BPC] = _extract(res.results[i]["raw"])
    return out


# revision 8
# speedup vs baseline: 128.5475x; 128.5475x over previous
"""FlowNet correlation (max_disp=20, stride2=2 -> 441 channels) on 8 TRN2 cores.

Strategy
--------
Data-parallel over batch: B=16 -> 2 batches per core.

Per (batch, parity-class (y%2, x%2)) the correlation decomposes into an
independent sub-problem on a 32x48 pixel grid with 21x21 displacements into a
zero-padded 52x68 source grid:

    out[dy,dx,ye,xe] = sum_c f1s[c,ye,xe] * f2s[c, ye+dy, xe+dx]

This is computed on the PE array as banded all-pairs matmuls: stationary
operand = a block of 8x16 = 128 pixel columns of f1 (bf16), moving operand =
the (8+20)x(16+20) = 28x36 = 1008 source columns of padded f2 (bf16), PSUM
accumulates over the two 128-channel chunks.  Each output pixel's 441 useful
dot products sit in a 21x21 sub-rectangle of its PSUM row (43.75% PE
utilization).  The full band tiles are scaled by 1/C, cast to bf16, copied
PSUM->SBUF (split across Vector + Scalar engines), and DMAed contiguously to
DRAM.  The final 21x21-per-pixel shear extraction is a zero-copy
numpy as_strided view on the host, followed by one big transpose/copy.

Parities are handled with strided access patterns directly on unsplit SBUF
tensors (f1 [C,64,96], padded f2 [C,104,136]) - no physical parity split.
"""

import sys

for _p in ("/opt/trn_rl_repo", "/root/.axon_site/_ro/trn_rl_repo"):
    if _p not in sys.path:
        sys.path.insert(0, _p)

import numpy as np

import concourse.bass as bass
import concourse.bacc as bacc
import concourse.mybir as mybir
import concourse.tile as tile
from concourse.bass_utils import run_bass_kernel_spmd

# Problem constants (hardcoded per contract)
B, C, H, W = 16, 256, 64, 96
NCORES = 8
BPC = B // NCORES            # batches per core
D = 21                       # displacements per axis
PAD_V, PAD_U = 52, 68        # padded source subgrid (per parity class)
P, Q = 8, 16                 # pixel block (y_e x x_e); P*Q = 128 stationary cols
SV, SU = P + 20, Q + 20      # source block 28 x 36 = 1008 moving cols
NIY, NIX = 32 // P, 48 // Q  # 4 x 3 blocks per class
F32 = mybir.dt.float32
BF16 = mybir.dt.bfloat16

_NC_CACHE = {}


def _build_nc(rep: int = 1):
    """Build the SPMD bass program (identical on all 8 cores).

    rep > 1 repeats the compute+output loop (same outputs) for timing slopes.
    """
    nc = bacc.Bacc("TRN2", target_bir_lowering=False, debug=False)
    f1d = nc.declare_dram_parameter("input1", [BPC, C, H, W], F32, isOutput=False)
    f2d = nc.declare_dram_parameter("input2", [BPC, C, H, W], F32, isOutput=False)
    # raw band tiles: [b, cls, iy, ix, 128 pixel, 28*36 source] bf16
    raw = nc.declare_dram_parameter(
        "raw", [BPC, 4, NIY, NIX, P * Q, SV * SU], BF16, isOutput=True
    )

    with tile.TileContext(nc) as tc:
        with (
            tc.tile_pool(name="f1p", bufs=1) as f1pool,
            tc.tile_pool(name="f2p", bufs=1) as f2pool,
            tc.tile_pool(name="stage", bufs=4) as spool,
            tc.tile_pool(name="psum", bufs=4, space="PSUM") as ppool,
        ):
            # f1 raw (single-buffered; only read by the staging copies)
            f1t = [f1pool.tile([128, H, W], BF16, name=f"f1r_{k}", tag=f"f1r_{k}")
                   for k in range(2)]
            # f1 staged block-major [ix, 128 pixel cols]; one tile per
            # (buf, chunk, cls, iy) so each matmul has a single staging producer
            f1b = [[[[f1pool.tile([128, NIX, P * Q], BF16,
                                  name=f"f1b_{i}_{k}_{c}_{j}",
                                  tag=f"f1b_{i}_{k}_{c}_{j}")
                      for j in range(NIY)] for c in range(4)]
                    for k in range(2)] for i in range(2)]
            f2t = [[f2pool.tile([128, 104, 136], BF16, name=f"f2_{i}_{k}", tag=f"f2_{i}_{k}")
                    for k in range(2)] for i in range(2)]

            # zero only the pad borders of f2 (stay zero forever; interior
            # is rewritten by every load) - on gpsimd to keep DVE free
            for i in range(2):
                for k in range(2):
                    t = f2t[i][k]
                    nc.gpsimd.memset(t[:, 0:20, :], 0.0)
                    nc.gpsimd.memset(t[:, 84:104, :], 0.0)
                    nc.gpsimd.memset(t[:, 20:84, 0:20], 0.0)
                    nc.gpsimd.memset(t[:, 20:84, 116:136], 0.0)

            def load_b(b):
                buf = b % 2
                for k in range(2):  # channel chunk
                    c0 = k * 128
                    nc.gpsimd.dma_start(
                        out=f1t[k][:], in_=f1d[b, c0:c0 + 128]
                    )
                    nc.gpsimd.dma_start(
                        out=f2t[buf][k][:, 20:84, 20:116],
                        in_=f2d[b, c0:c0 + 128],
                    )
                # stage f1 into contiguous 128-column pixel blocks
                # (matmul stationary APs must have a single free dim)
                for k in range(2):
                    f1vk = f1t[k][:].rearrange(
                        "p (y a) (x c) -> p a c y x", a=2, c=2)
                    for cls in range(4):
                        py, px = cls >> 1, cls & 1
                        for iy in range(NIY):
                            nc.gpsimd.tensor_copy(
                                f1b[buf][k][cls][iy][:]
                                .rearrange("p i (y x) -> p i y x", y=P),
                                f1vk[:, py, px, iy * P:(iy + 1) * P, :]
                                .rearrange("p y (i x) -> p i y x", i=NIX),
                            )

            load_b(0)
            if BPC > 1:
                load_b(1)

            inv_c = 1.0 / C

            import contextlib

            def body():
                for b in range(BPC):
                    buf = b % 2
                    # parity-interleave views: [p, py, px, ve, ue]
                    f2v = [f2t[buf][k][:].rearrange(
                        "p (v a) (u c) -> p a c v u", a=2, c=2) for k in range(2)]
                    for cls in range(4):
                        py, px = cls >> 1, cls & 1
                        for iy in range(NIY):
                            y0 = iy * P
                            for ix in range(NIX):
                                x0 = ix * Q
                                ps = [ppool.tile([128, SV // 2, SU], F32, name=f"ps{h}",
                                                 tag=f"ps{h}") for h in range(2)]
                                for h in range(2):  # v-halves of moving operand
                                    v0 = y0 + h * (SV // 2)
                                    for k in range(2):  # channel chunk
                                        nc.tensor.matmul(
                                            ps[h][:],
                                            f1b[buf][k][cls][iy][:, ix],
                                            f2v[k][:, py, px,
                                                   v0:v0 + SV // 2,
                                                   x0:x0 + SU],
                                            start=(k == 0),
                                            stop=(k == 1),
                                        )
                                st = spool.tile([128, SV * SU], BF16, name="st", tag="st")
                                half = (SV // 2) * SU
                                # split the scale+cast copy across two engines
                                nc.vector.tensor_scalar_mul(
                                    st[:, 0:half],
                                    ps[0][:].rearrange("p a b -> p (a b)"),
                                    inv_c,
                                )
                                nc.scalar.activation(
                                    st[:, half:2 * half],
                                    ps[1][:].rearrange("p a b -> p (a b)"),
                                    mybir.ActivationFunctionType.Copy,
                                    scale=inv_c,
                                )
                                nc.sync.dma_start(
                                    out=raw[b, cls, iy, ix], in_=st[:]
                                )

            if rep >= 1:
                for r in range(rep):
                    body()
            else:
                with tc.For_i(0, -rep, 1):
                    body()
    nc.compile()
    return nc


def _get_nc(rep: int = 1):
    if rep not in _NC_CACHE:
        _NC_CACHE[rep] = _build_nc(rep)
    return _NC_CACHE[rep]


def _extract(raw_np: np.ndarray) -> np.ndarray:
    """Host-side shear extraction: raw [BPC,4,NIY,NIX,128,1008] -> [BPC,441,64,96]."""
    arr = np.ascontiguousarray(raw_np).astype(np.float32)
    V = arr.reshape(BPC, 4, NIY, NIX, P, Q, SV, SU)
    s = V.strides
    Wv = np.lib.stride_tricks.as_strided(
        V,
        shape=(BPC, 4, NIY, NIX, P, Q, D, D),
        strides=(s[0], s[1], s[2], s[3], s[4] + s[6], s[5] + s[7], s[6], s[7]),
    )
    # -> [b, cls, dy, dx, iy, yo, ix, xo] -> [b, cls, 441, 32, 48]
    T = Wv.transpose(0, 1, 6, 7, 2, 4, 3, 5).reshape(BPC, 4, D * D, 32, 48)
    out = np.empty((BPC, D * D, H, W), np.float32)
    for cls in range(4):
        py, px = cls >> 1, cls & 1
        out[:, :, py::2, px::2] = T[:, cls]
    return out


def kernel(input1: np.ndarray, input2: np.ndarray) -> np.ndarray:
    input1 = np.ascontiguousarray(input1, dtype=np.float32)
    input2 = np.ascontiguousarray(input2, dtype=np.float32)
    nc = _get_nc(1)
    in_maps = [
        {"input1": input1[i * BPC:(i + 1) * BPC],
         "input2": input2[i * BPC:(i + 1) * BPC]}
        for i in range(NCORES)
    ]
    res = run_bass_kernel_spmd(nc, in_maps, list(range(NCORES)))
    out = np.empty((B, D * D, H, W), np.float32)
    for i in range(NCORES):
        out[i * BPC:(i + 1) * BPC] = _extract(res.results[i]["raw"])
    return out
